# revision 8
# baseline (speedup 1.0000x reference)
"""AttentionBlock Bass kernel for TRN2 — per-core program builder (v6).

Per core: 2 batches of x [512, 1024] (C=512 channels, T=1024 spatial).
Pipeline: layernorm (spatial) -> qkv 1x1 conv -> 8-head attention -> proj
-> residual add.  Matmuls in bf16 (1 cyc/row), accumulation fp32 in PSUM.

The softmax exp chain on the scalar/ACT engine (128 ACTIVATEs of
[128, 1024] ~= 142us) is the pacing critical path; everything else is
arranged to keep it dense:
  - a dummy exp right after the constant memsets forces the ACT exp
    table load into the DMA shadow instead of mid-kernel.
  - qkv units for the first head pair (ot 0 and 4) run before the pair
    loop; all other qkv/vt/LN units are fillers inside the loop.
  - QK pair matmuls emitted interleaved (A0,B0,A1,B1) at base
    partitions 0/64 (disjoint PE row groups) and, with their exps,
    under tc.high_priority so the scheduler never lets filler matmuls
    delay the next exp.
  - PSUM: one shared 3-buffer pool of [128,1024] tiles serves the QK
    pairs AND the filler units (qkv/vt/proj), so the next pair's QK can
    start before the previous pair's last exp completes; AV accumulates
    in two [65,512] half-tiles (1 bank each).
  - softmax denominator folded into AV via a ones-column on v^T (M=65);
    normalization per t-half: psum row -> reciprocal -> gpsimd
    partition-broadcast -> single multiply reading a' straight from
    PSUM (no staging copy).
  - LN uses bn_stats/bn_aggr on DVE (no Square pass on the ACT engine).
  - drain: proj(b1) units pre-accumulate c-chunks 0..2 while the last
    heads' AV runs; the c=3 matmuls are split into 64-contraction
    halves (heads 6 rows / head 7 rows) so after each final norm only
    a handful of matmuls + the evac trail, and the PE never idles past
    the HAM window.

Host-side layouts (see shard_inputs):
  x/out DRAM  [2*512, 1024]   row = b*512 + c
  wqkvT DRAM  [512, 1536]     bf16, output channels permuted head-major
                              q_all|k_all|v_all (qkv_perm)
  bq DRAM     [128, 8]        f32, q|k bias columns per 128-row tile
  bv DRAM     [1, 512]        bf16, v bias row (head-major)
  wprojT DRAM [512, 512]      bf16
  bp DRAM     [128, 4]        f32
"""

import numpy as np
from contextlib import ExitStack

import concourse.bass as bass
import concourse.mybir as mybir
from concourse.bacc import Bacc
from concourse.tile import TileContext
from bass_rust import ScopedClock

F32 = mybir.dt.float32
BF16 = mybir.dt.bfloat16
AF = mybir.ActivationFunctionType
ALU = mybir.AluOpType
AX = mybir.AxisListType

P = 128
T = 1024
NB = 2
C = 512
NH = 8
CH = 64
KC = C // P         # 4 contraction chunks
NQK = (2 * C) // P  # 8 q|k output tiles
EPS = 1e-5
VW = CH + 1         # per-head v^T block width (ones column folded in)
HPRI = 50000        # priority offset for the QK/exp critical chain


class SplitDrainTileContext(TileContext):
    """Kernel-tail drain split into 1-wait chunks (this walrus rejects >1
    sync wait per SP CTRL instruction)."""

    def _drain_and_barrier(self, tick_clock, wait_clock):
        drain_inst = self.nc.sync.drain()
        wait_clock.add_sem_waits(
            drain_inst.ins, ScopedClock({None: tick_clock.global_clock})
        )
        si = drain_inst.ins.sync_info
        waits = list(si.on_wait) if si and si.on_wait else []
        if len(waits) > 1:
            si.on_wait = waits[:1]
            for w in waits[1:]:
                extra = self.nc.sync.drain()
                if extra.ins.sync_info is None:
                    extra.ins.sync_info = mybir.SyncInfo(on_wait=[], on_update=[])
                extra.ins.sync_info.on_wait = [w]

        self.nc.all_engine_barrier()
        assert self.sems is not None
        popped = self.nc._tile_sem_poison_stack.pop()
        assert popped is self._sem_poison
        self.nc.clear_and_free_semaphores(list(self.sems.allocated().values()))
        self.nc.all_engine_barrier()


def build_nc(with_bias=False, debug=False) -> bass.Bass:
    nc = Bacc()
    x = nc.declare_dram_parameter("x", [NB * C, T], F32, isOutput=False)
    wqkvT = nc.declare_dram_parameter("wqkvT", [C, 3 * C], BF16, isOutput=False)
    wprojT = nc.declare_dram_parameter("wprojT", [C, C], BF16, isOutput=False)
    bq = nc.declare_dram_parameter("bq", [P, NQK], F32, isOutput=False)
    bv = nc.declare_dram_parameter("bv", [1, C], BF16, isOutput=False)
    bp = nc.declare_dram_parameter("bp", [P, KC], F32, isOutput=False)
    out = nc.declare_dram_parameter("out", [NB * C, T], F32, isOutput=True)

    with SplitDrainTileContext(nc) as tc, ExitStack() as ctx:
        const = ctx.enter_context(tc.tile_pool(name="const", bufs=1))
        xin = ctx.enter_context(tc.tile_pool(name="xin", bufs=4))
        stat = ctx.enter_context(tc.tile_pool(name="stat", bufs=8))
        xnbp = ctx.enter_context(tc.tile_pool(name="xnb", bufs=2 * KC))
        qkvp = ctx.enter_context(tc.tile_pool(name="qkv", bufs=2 * NQK))
        vtp = ctx.enter_context(tc.tile_pool(name="vt", bufs=16))
        wexpp = ctx.enter_context(tc.tile_pool(name="wexp", bufs=18))
        aallp = ctx.enter_context(tc.tile_pool(name="aall", bufs=2 * KC))
        rbp = ctx.enter_context(tc.tile_pool(name="rb", bufs=4))
        drp = ctx.enter_context(tc.tile_pool(name="dr", bufs=8))
        outp = ctx.enter_context(tc.tile_pool(name="outp", bufs=2))

        # shared psum pool: QK pair tiles + filler unit psums (3 x 2 banks)
        qkf = ctx.enter_context(tc.tile_pool(name="qkf", bufs=3, space="PSUM"))
        # AV accumulates per t-half (1 bank each)
        av_ps = ctx.enter_context(tc.tile_pool(name="avps", bufs=2, space="PSUM"))

        # ---- input DMA: x(b0) chunks interleaved with wqkv chunks so the
        # LN pipeline and the first qkv units both start early ----
        xts = {}
        wq_t = [None] * KC

        def dma_x(b, c):
            xt = xin.tile([P, T], F32, tag="xin", name=f"xin_{b}_{c}")
            nc.sync.dma_start(
                out=xt[:], in_=x[b * C + c * P : b * C + (c + 1) * P, :]
            )
            xts[(b, c)] = xt

        def dma_wq(c):
            t_ = const.tile([P, 3 * C], BF16, tag=f"wq{c}", name=f"wq{c}")
            nc.sync.dma_start(out=t_[:], in_=wqkvT[c * P : (c + 1) * P, :])
            wq_t[c] = t_

        dma_x(0, 0)
        dma_wq(0)
        dma_x(0, 1)
        dma_x(0, 2)
        dma_wq(1)
        dma_x(0, 3)
        dma_wq(2)
        dma_wq(3)
        if with_bias:
            bq_t = const.tile([P, NQK], F32, tag="bq")
            nc.sync.dma_start(out=bq_t[:], in_=bq[:])
            bv_t = const.tile([1, C], BF16, tag="bv")
            nc.sync.dma_start(out=bv_t[:], in_=bv[:])
        for c in range(KC):
            dma_x(1, c)
        wp_t = []
        for c in range(KC):
            t_ = const.tile([P, C], BF16, tag=f"wp{c}", name=f"wp{c}")
            nc.sync.dma_start(out=t_[:], in_=wprojT[c * P : (c + 1) * P, :])
            wp_t.append(t_)
        if with_bias:
            bp_t = const.tile([P, KC], F32, tag="bp")
            nc.sync.dma_start(out=bp_t[:], in_=bp[:])
        eps_t = const.tile([P, 1], F32, tag="eps")
        nc.gpsimd.memset(eps_t[:], EPS)
        ones_t = const.tile([P, 8], BF16, tag="ones")
        nc.gpsimd.memset(ones_t[:], 1.0)
        if with_bias:
            onerow_t = const.tile([1, P], BF16, tag="onerow")
            nc.gpsimd.memset(onerow_t[:], 1.0)
        # dummy exp: forces the ACT exp-table load into the input-DMA shadow
        warm_t = stat.tile([P, 1], F32, tag="warm")
        nc.scalar.activation(warm_t[:], eps_t[:], AF.Exp)

        def head_slice(tiles, h):
            off = (h % 2) * CH
            return tiles[h // 2][off : off + CH, :]

        # per-batch state
        xnb_t = [[None] * KC for _ in range(NB)]
        qkv_t = [[None] * NQK for _ in range(NB)]
        vt_t = [[None] * 8 for _ in range(NB)]
        aall_t = [[None] * KC for _ in range(NB)]
        wexp_t = {}   # (b, h) -> list of 8 chunk tiles
        av_tiles = {}  # (b, h, half) -> [65, 512] psum tile

        def emit_ln(b, c):
            xt = xts[(b, c)]
            bns = stat.tile([P, 12], F32, tag="bns", name=f"bns_{b}_{c}")
            for k in range(2):
                nc.vector.bn_stats(
                    bns[:, 6 * k : 6 * (k + 1)], xt[:, 512 * k : 512 * (k + 1)]
                )
            mv = stat.tile([P, 2], F32, tag="mv", name=f"mv_{b}_{c}")
            nc.vector.bn_aggr(mv[:], bns[:])
            std = stat.tile([P, 1], F32, tag="std", name=f"std_{b}_{c}")
            nc.scalar.activation(std[:], mv[:, 1:2], AF.Sqrt, bias=eps_t[:])
            rstd = stat.tile([P, 1], F32, tag="rstd", name=f"rstd_{b}_{c}")
            nc.vector.reciprocal_approx_fast(rstd[:], std[:])
            xnb = xnbp.tile([P, T], BF16, tag="xnb", name=f"xnb_{b}_{c}")
            nc.vector.tensor_scalar(
                xnb[:], xt[:], scalar1=mv[:, 0:1], scalar2=rstd[:],
                op0=ALU.subtract, op1=ALU.mult,
            )
            xnb_t[b][c] = xnb

        def emit_vt_unit(b, s):
            """v^T for spatial chunk s, all 8 heads: [128 t, 8*65] bf16
            with per-head ones columns."""
            pst = qkf.tile([P, T], F32, tag="qk", name=f"vps_{b}_{s}")
            ps = pst[:, 0:C]
            for c in range(KC):
                nc.tensor.matmul(
                    ps,
                    xnb_t[b][c][:, s * P : (s + 1) * P],
                    wq_t[c][:, 2 * C : 3 * C],
                    start=(c == 0),
                    stop=(c == KC - 1) and not with_bias,
                )
            if with_bias:
                nc.tensor.matmul(ps, onerow_t[:], bv_t[:], start=False, stop=True)
            vt = vtp.tile([P, 8 * VW], BF16, tag="vt", name=f"vt_{b}_{s}")
            nc.vector.tensor_copy(
                vt[:].rearrange("p (h c) -> p h c", c=VW)[:, :, 0:CH],
                ps.rearrange("p (h c) -> p h c", c=CH),
            )
            nc.vector.tensor_copy(
                vt[:].rearrange("p (h c) -> p h c", c=VW)[:, :, CH : CH + 1],
                ones_t[:].rearrange("p (h c) -> p h c", c=1),
            )
            vt_t[b][s] = vt

        def emit_qkv_unit(b, ot):
            """One q|k output tile [128, T]: 8 matmuls + one evac pass."""
            qt = qkvp.tile([P, T], BF16, tag="qkv", name=f"qkv_{b}_{ot}")
            ps = qkf.tile([P, T], F32, tag="qk", name=f"qps_{b}_{ot}")
            for c in range(KC):
                for half in range(2):
                    nc.tensor.matmul(
                        ps[:, half * 512 : (half + 1) * 512],
                        wq_t[c][:, ot * P : (ot + 1) * P],
                        xnb_t[b][c][:, half * 512 : (half + 1) * 512],
                        start=(c == 0),
                        stop=(c == KC - 1),
                    )
            if with_bias:
                nc.vector.tensor_scalar(
                    qt[:], ps[:], scalar1=bq_t[:, ot : ot + 1], scalar2=None,
                    op0=ALU.add,
                )
            else:
                nc.vector.tensor_copy(qt[:], ps[:])
            qkv_t[b][ot] = qt

        def emit_qk_pair(b, hA, s):
            """scores chunk s for heads hA/hA+1: matmuls interleaved
            (A0,B0,A1,B1) in disjoint PE row groups; one exp per head.
            Runs at high priority — this is the exp-chain feeder."""
            hB = hA + 1
            q_all, k_all = qkv_t[b][0:4], qkv_t[b][4:8]
            qA, kA = head_slice(q_all, hA), head_slice(k_all, hA)
            qB, kB = head_slice(q_all, hB), head_slice(k_all, hB)
            with tc.high_priority(offset=HPRI):
                pA = qkf.tile([P, T], F32, tag="qk", name=f"qk_{b}_{hA}_{s}")
                pB = qkf.tile([P, T], F32, tag="qk", name=f"qk_{b}_{hB}_{s}")
                for half in range(2):
                    sl = slice(half * 512, (half + 1) * 512)
                    nc.tensor.matmul(
                        pA[:, sl], kA[:, s * P : (s + 1) * P], qA[:, sl],
                        start=True, stop=True,
                    )
                    nc.tensor.matmul(
                        pB[:, sl], kB[:, s * P : (s + 1) * P], qB[:, sl],
                        start=True, stop=True,
                    )
                for h, ps in ((hA, pA), (hB, pB)):
                    we = wexpp.tile(
                        [P, T], BF16, tag="wexp", name=f"we_{b}_{h}_{s}"
                    )
                    nc.scalar.activation(we[:], ps[:], AF.Exp, scale=0.125)
                    wexp_t.setdefault((b, h), []).append(we)

        def head_off(h):
            return h * VW

        def emit_av_q(b, h, half, squad):
            """AV quad: 4 accumulation matmuls for (head, t-half),
            s-chunks 4*squad..4*squad+3."""
            if squad == 0:
                av_tiles[(b, h, half)] = av_ps.tile(
                    [VW, 512], F32, tag="av", name=f"av_{b}_{h}_{half}"
                )
            av = av_tiles[(b, h, half)]
            for sp in range(4 * squad, 4 * squad + 4):
                nc.tensor.matmul(
                    av[:],
                    vt_t[b][sp][:, head_off(h) : head_off(h) + VW],
                    wexp_t[(b, h)][sp][:, half * 512 : (half + 1) * 512],
                    start=(sp == 0),
                    stop=(sp == 7),
                )

        def emit_norm_half(b, h, half):
            """softmax normalization for one t-half: denom row out of psum,
            reciprocal, gpsimd broadcast, then one multiply reading a'
            straight from PSUM into the aall slice (releases the bank)."""
            av = av_tiles[(b, h, half)]
            draw = drp.tile([1, 512], F32, tag="draw", name=f"dw_{b}_{h}_{half}")
            nc.vector.tensor_copy(draw[:], av[CH : CH + 1, :])
            drow = drp.tile([1, 512], F32, tag="dr", name=f"dr_{b}_{h}_{half}")
            nc.vector.reciprocal_approx_fast(drow[:], draw[:])
            rb = rbp.tile([CH, 512], F32, tag="rb", name=f"rb_{b}_{h}_{half}")
            nc.gpsimd.partition_broadcast(rb[:], drow[:])
            if aall_t[b][0] is None:
                for i in range(KC):
                    aall_t[b][i] = aallp.tile(
                        [P, T], BF16, tag="aall", name=f"aall_{b}_{i}"
                    )
            dest = head_slice(aall_t[b], h)[:, half * 512 : (half + 1) * 512]
            nc.vector.tensor_tensor(dest, av[0:CH, :], rb[:], op=ALU.mult)
            if half == 1:
                del wexp_t[(b, h)]

        def proj_evac_half(b, ot, ps_half, half):
            o_t = outp.tile([P, 512], F32, tag="outp", name=f"out_{b}_{ot}_{half}")
            sl = slice(half * 512, (half + 1) * 512)
            if with_bias:
                nc.vector.tensor_scalar(
                    o_t[:], ps_half, scalar1=bp_t[:, ot : ot + 1], scalar2=None,
                    op0=ALU.add,
                )
                nc.vector.tensor_tensor(
                    o_t[:], o_t[:], xnb_t[b][ot][:, sl], op=ALU.add
                )
            else:
                nc.vector.scalar_tensor_tensor(
                    o_t[:], ps_half, 1.0, xnb_t[b][ot][:, sl],
                    op0=ALU.mult, op1=ALU.add,
                )
            nc.sync.dma_start(
                out=out[b * C + ot * P : b * C + (ot + 1) * P, sl], in_=o_t[:]
            )

        def proj_cmms(b, ot, pss, cs, start, stop, rows=None):
            """proj matmuls for chunks cs into pss (2 halves); rows
            optionally restricts the contraction to a 64-row slice."""
            r = rows if rows is not None else slice(0, P)
            for c in cs:
                for half in range(2):
                    nc.tensor.matmul(
                        pss[half],
                        wp_t[c][r, ot * P : (ot + 1) * P],
                        aall_t[b][c][r, half * 512 : (half + 1) * 512],
                        start=start and c == cs[0],
                        stop=stop and c == cs[-1],
                    )

        def emit_proj_unit(b, ot):
            pst = qkf.tile([P, T], F32, tag="qk", name=f"prj_{b}_{ot}")
            pss = [pst[:, 0:512], pst[:, 512:T]]
            proj_cmms(b, ot, pss, list(range(KC)), start=True, stop=True)
            for half in range(2):
                proj_evac_half(b, ot, pss[half], half)

        # ---------------- pipelined schedule ----------------
        for c in range(KC):
            emit_ln(0, c)
        # only the first pair's q|k tiles before the loop
        emit_qkv_unit(0, 0)
        emit_qkv_unit(0, 4)

        fillers = (
            [("qkv", 0, 1), ("qkv", 0, 5)]
            + [("vt", 0, s) for s in range(8)]
            + [("qkv", 0, 2), ("qkv", 0, 6), ("qkv", 0, 3), ("qkv", 0, 7)]
            + [("ln", 1, c) for c in range(KC)]
            + [("qkv", 1, 0), ("qkv", 1, 4), ("qkv", 1, 1), ("qkv", 1, 5),
               ("qkv", 1, 2), ("qkv", 1, 6), ("qkv", 1, 3), ("qkv", 1, 7)]
            + [("vt", 1, s) for s in range(8)]
        )
        proj_units = [(0, ot) for ot in range(KC)]

        def pop_filler(allow_proj):
            if fillers:
                kind, fb, fo = fillers.pop(0)
                if kind == "ln":
                    emit_ln(fb, fo)
                elif kind == "vt":
                    emit_vt_unit(fb, fo)
                else:
                    emit_qkv_unit(fb, fo)
                return True
            if allow_proj and proj_units:
                pb, po = proj_units.pop(0)
                emit_proj_unit(pb, po)
                return True
            return False

        # AV weave for the previous pair: head A's 4 quads over s=0..3
        # (norm per half as it completes), head B over s=4..7.
        def weave_av(pb, pA, s):
            h = pA if s < 4 else pA + 1
            sq = s % 4
            emit_av_q(pb, h, sq // 2, sq % 2)
            if sq == 1:
                emit_norm_half(pb, h, 0)
            elif sq == 3:
                emit_norm_half(pb, h, 1)

        pairs = [(b, 2 * i) for b in range(NB) for i in range(NH // 2)]
        prevp = None
        for pi, (b, hA) in enumerate(pairs):
            for s in range(8):
                emit_qk_pair(b, hA, s)
                if prevp is not None:
                    if pi <= 4 and s in (0, 1, 2, 3, 5, 7):
                        pop_filler(allow_proj=False)
                    elif pi >= 5 and s in (1, 5):
                        pop_filler(allow_proj=False)
                    weave_av(prevp[0], prevp[1], s)
                    if pi >= 5 and s in (2, 6):
                        pop_filler(allow_proj=True)
                else:
                    pop_filler(allow_proj=False)
                    if s >= 6:
                        pop_filler(allow_proj=False)
            prevp = (b, hA)

        # ---------------- drain ----------------
        # AV + norm for the last pair (b1 heads 6,7) with proj(b1) woven in:
        # ot0..2 pre-accumulate c0..2 in the freed shared-pool tiles; their
        # c3 matmuls split into head-6-rows / head-7-rows pieces that chase
        # the final norms; ot3 rides the freed AV banks per half.
        pb, pA = prevp
        h6, h7 = pA, pA + 1
        pre = {}

        def proj_pre(ot):
            pst = qkf.tile([P, T], F32, tag="qk", name=f"prj1_{ot}")
            pss = [pst[:, 0:512], pst[:, 512:T]]
            proj_cmms(1, ot, pss, [0, 1, 2], start=True, stop=False)
            pre[ot] = pss

        def c3_piece(ot, half, rows, stop):
            nc.tensor.matmul(
                pre[ot][half],
                wp_t[3][rows, ot * P : (ot + 1) * P],
                aall_t[1][3][rows, half * 512 : (half + 1) * 512],
                start=False,
                stop=stop,
            )

        while fillers or proj_units:
            pop_filler(allow_proj=True)

        emit_av_q(1, h6, 0, 0); proj_pre(0)
        emit_av_q(1, h6, 0, 1); emit_norm_half(1, h6, 0); proj_pre(1)
        emit_av_q(1, h6, 1, 0); proj_pre(2)
        emit_av_q(1, h6, 1, 1); emit_norm_half(1, h6, 1)

        emit_av_q(1, h7, 0, 0)
        for ot in (0, 1, 2):   # head-6 rows, half 0 (needs norm(6,0))
            c3_piece(ot, 0, slice(0, CH), stop=False)
        emit_av_q(1, h7, 0, 1)
        emit_norm_half(1, h7, 0)
        for ot in (0, 1, 2):   # head-6 rows, half 1
            c3_piece(ot, 1, slice(0, CH), stop=False)
        # ot3 half0 rides the AV bank freed by norm(7,0)
        ps3 = [None, None]
        ps3[0] = av_ps.tile([P, 512], F32, tag="av", name="prj3_0")
        for c in range(3):
            nc.tensor.matmul(
                ps3[0], wp_t[c][:, 3 * P : 4 * P],
                aall_t[1][c][:, 0:512], start=(c == 0), stop=False,
            )
        emit_av_q(1, h7, 1, 0)
        for ot in (0, 1, 2):   # head-7 rows, half 0 (needs norm(7,0))
            c3_piece(ot, 0, slice(CH, P), stop=True)
            proj_evac_half(1, ot, pre[ot][0], 0)
        nc.tensor.matmul(      # ot3 half0 c3 (full contraction)
            ps3[0], wp_t[3][:, 3 * P : 4 * P],
            aall_t[1][3][:, 0:512], start=False, stop=True,
        )
        proj_evac_half(1, 3, ps3[0], 0)
        emit_av_q(1, h7, 1, 1)
        emit_norm_half(1, h7, 1)
        ps3[1] = av_ps.tile([P, 512], F32, tag="av", name="prj3_1")
        for c in range(3):
            nc.tensor.matmul(
                ps3[1], wp_t[c][:, 3 * P : 4 * P],
                aall_t[1][c][:, 512:T], start=(c == 0), stop=False,
            )
        for ot in (0, 1, 2):   # head-7 rows, half 1 — the true tail
            c3_piece(ot, 1, slice(CH, P), stop=True)
            proj_evac_half(1, ot, pre[ot][1], 1)
        nc.tensor.matmul(
            ps3[1], wp_t[3][:, 3 * P : 4 * P],
            aall_t[1][3][:, 512:T], start=False, stop=True,
        )
        proj_evac_half(1, 3, ps3[1], 1)

    nc.finalize()
    return nc


def qkv_perm():
    """Output-channel permutation: legacy [h][q|k|v] interleave -> head-major
    q_all (512) | k_all (512) | v_all (512)."""
    idx = []
    for part in range(3):
        for h in range(NH):
            idx.append(192 * h + part * CH + np.arange(CH))
    return np.concatenate(idx)


def shard_inputs(x, w_qkv, b_qkv, w_proj, b_proj, n_cores=8):
    """Full inputs -> per-core in_maps."""
    import ml_dtypes

    perm = qkv_perm()
    xr = np.ascontiguousarray(x.reshape(16, C, T), dtype=np.float32)
    wqkvT = np.ascontiguousarray(w_qkv[perm].T.astype(ml_dtypes.bfloat16))
    wprojT = np.ascontiguousarray(w_proj.T.astype(ml_dtypes.bfloat16))
    bqp = np.asarray(b_qkv)[perm]
    bqm = np.ascontiguousarray(bqp[: 2 * C].reshape(NQK, P).T, dtype=np.float32)
    bvm = np.ascontiguousarray(bqp[2 * C :].reshape(1, C).astype(ml_dtypes.bfloat16))
    bpm = np.ascontiguousarray(np.asarray(b_proj).reshape(KC, P).T, dtype=np.float32)
    in_maps = []
    for i in range(n_cores):
        in_maps.append(
            {
                "x": np.ascontiguousarray(xr[NB * i : NB * (i + 1)].reshape(NB * C, T)),
                "wqkvT": wqkvT,
                "wprojT": wprojT,
                "bq": bqm,
                "bv": bvm,
                "bp": bpm,
            }
        )
    return in_maps


def gather_outputs(results, n_cores=8):
    outs = [results[i]["out"].reshape(NB, C, 32, 32) for i in range(n_cores)]
    return np.concatenate(outs, axis=0)


# ---------------------------------------------------------------------------
# Cached 8-core PJRT executor (mirrors concourse.bass2jax.run_bass_via_pjrt,
# but keeps the jitted sharded callable alive so repeat kernel() calls skip
# retracing/recompiling)
# ---------------------------------------------------------------------------
import jax
from jax.sharding import Mesh, PartitionSpec

from concourse import bass2jax


def _shard_map():
    try:
        from jax.experimental.shard_map import shard_map
        return shard_map
    except ImportError:
        from jax.experimental import shard_map as sm
        return sm.shard_map


class _Runner:
    def __init__(self, nc, n_cores=8):
        bass2jax.install_neuronx_cc_hook()
        self.nc = nc
        self.n_cores = n_cores
        partition_name = (
            nc.partition_id_tensor.name if nc.partition_id_tensor else None
        )
        in_names, out_names, out_avals, zero_outs = [], [], [], []
        for alloc in nc.m.functions[0].allocations:
            if not isinstance(alloc, mybir.MemoryLocationSet):
                continue
            name = alloc.memorylocations[0].name
            if alloc.kind == "ExternalInput":
                if name != partition_name:
                    in_names.append(name)
            elif alloc.kind == "ExternalOutput":
                shape = tuple(alloc.tensor_shape)
                dtype = mybir.dt.np(alloc.dtype)
                out_names.append(name)
                out_avals.append(jax.core.ShapedArray(shape, dtype))
                zero_outs.append(np.zeros(shape, dtype))
        self.n_params = len(in_names)
        self.out_names = out_names
        self.out_avals = out_avals
        self.zero_outs = zero_outs
        n_outs = len(out_avals)
        in_names = in_names + out_names
        if partition_name is not None:
            in_names.append(partition_name)
        self.in_names = in_names

        def _body(*args):
            operands = list(args)
            if partition_name is not None:
                operands.append(bass2jax.partition_id_tensor())
            outs = bass2jax._bass_exec_p.bind(
                *operands,
                out_avals=tuple(out_avals),
                in_names=tuple(in_names),
                out_names=tuple(out_names),
                lowering_input_output_aliases=(),
                sim_require_finite=True,
                sim_require_nnan=True,
                nc=nc,
            )
            return tuple(outs)

        devices = jax.devices()[:n_cores]
        self.mesh = Mesh(np.asarray(devices), ("core",))
        shard_map = _shard_map()
        in_specs = (PartitionSpec("core"),) * (self.n_params + n_outs)
        out_specs = (PartitionSpec("core"),) * n_outs
        self.sharded = jax.jit(
            shard_map(
                _body,
                mesh=self.mesh,
                in_specs=in_specs,
                out_specs=out_specs,
                check_rep=False,
            ),
            keep_unused=True,
        )

    def run(self, in_maps):
        per_core = [
            [np.asarray(m[name]) for name in self.in_names[: self.n_params]]
            for m in in_maps
        ]
        concat_in = [
            np.concatenate([per_core[c][i] for c in range(self.n_cores)], axis=0)
            for i in range(self.n_params)
        ]
        concat_zeros = [
            np.zeros((self.n_cores * z.shape[0], *z.shape[1:]), z.dtype)
            for z in self.zero_outs
        ]
        out_arrs = self.sharded(*concat_in, *concat_zeros)
        jax.block_until_ready(out_arrs)
        return [
            {
                name: np.asarray(out_arrs[i]).reshape(
                    self.n_cores, *self.out_avals[i].shape
                )[c]
                for i, name in enumerate(self.out_names)
            }
            for c in range(self.n_cores)
        ]


_RUNNERS = {}


def _get_runner(with_bias=False):
    if with_bias not in _RUNNERS:
        _RUNNERS[with_bias] = _Runner(build_nc(with_bias=with_bias), 8)
    return _RUNNERS[with_bias]


def kernel(x, w_qkv, b_qkv, w_proj, b_proj):
    """Full-input AttentionBlock forward on 8 TRN2 NeuronCores.

    x [16, 512, 32, 32] f32 -> out [16, 512, 32, 32] f32.
    Data-parallel over batch: core i computes batches 2i, 2i+1.
    """
    with_bias = bool(np.any(np.asarray(b_qkv))) or bool(np.any(np.asarray(b_proj)))
    runner = _get_runner(with_bias)
    in_maps = shard_inputs(x, w_qkv, b_qkv, w_proj, b_proj, 8)
    results = runner.run(in_maps)
    return gather_outputs(results, 8).astype(np.float32)


# revision 10
# speedup vs baseline: 1.0148x; 1.0148x over previous
"""AttentionBlock Bass kernel for TRN2 — per-core program builder (v6).

Per core: 2 batches of x [512, 1024] (C=512 channels, T=1024 spatial).
Pipeline: layernorm (spatial) -> qkv 1x1 conv -> 8-head attention -> proj
-> residual add.  Matmuls in bf16 (1 cyc/row), accumulation fp32 in PSUM.

The softmax exp chain on the scalar/ACT engine (128 ACTIVATEs of
[128, 1024] ~= 142us) is the pacing critical path; everything else is
arranged to keep it dense:
  - a dummy exp right after the constant memsets forces the ACT exp
    table load into the DMA shadow instead of mid-kernel.
  - qkv units for the first head pair (ot 0 and 4) run before the pair
    loop; all other qkv/vt/LN units are fillers inside the loop.
  - QK pair matmuls emitted interleaved (A0,B0,A1,B1) at base
    partitions 0/64 (disjoint PE row groups) and, with their exps,
    under tc.high_priority so the scheduler never lets filler matmuls
    delay the next exp.
  - PSUM: one shared 3-buffer pool of [128,1024] tiles serves the QK
    pairs AND the filler units (qkv/vt/proj), so the next pair's QK can
    start before the previous pair's last exp completes; AV accumulates
    in two [65,512] half-tiles (1 bank each).
  - softmax denominator folded into AV via a ones-column on v^T (M=65);
    normalization per t-half: psum row -> reciprocal -> gpsimd
    partition-broadcast -> single multiply reading a' straight from
    PSUM (no staging copy).
  - LN uses bn_stats/bn_aggr on DVE (no Square pass on the ACT engine).
  - drain: proj(b1) units pre-accumulate c-chunks 0..2 while the last
    heads' AV runs; the c=3 matmuls are split into 64-contraction
    halves (heads 6 rows / head 7 rows) so after each final norm only
    a handful of matmuls + the evac trail, and the PE never idles past
    the HAM window.

Host-side layouts (see shard_inputs):
  x/out DRAM  [2*512, 1024]   row = b*512 + c
  wqkvT DRAM  [512, 1536]     bf16, output channels permuted head-major
                              q_all|k_all|v_all (qkv_perm)
  bq DRAM     [128, 8]        f32, q|k bias columns per 128-row tile
  bv DRAM     [1, 512]        bf16, v bias row (head-major)
  wprojT DRAM [512, 512]      bf16
  bp DRAM     [128, 4]        f32
"""

import numpy as np
from contextlib import ExitStack

import concourse.bass as bass
import concourse.mybir as mybir
from concourse.bacc import Bacc
from concourse.tile import TileContext
from bass_rust import ScopedClock

F32 = mybir.dt.float32
BF16 = mybir.dt.bfloat16
AF = mybir.ActivationFunctionType
ALU = mybir.AluOpType
AX = mybir.AxisListType

P = 128
T = 1024
NB = 2
C = 512
NH = 8
CH = 64
KC = C // P         # 4 contraction chunks
NQK = (2 * C) // P  # 8 q|k output tiles
EPS = 1e-5
VW = CH + 1         # per-head v^T block width (ones column folded in)
HPRI = 50000        # priority offset for the QK/exp critical chain


class SplitDrainTileContext(TileContext):
    """Kernel-tail drain split into 1-wait chunks (this walrus rejects >1
    sync wait per SP CTRL instruction)."""

    def _drain_and_barrier(self, tick_clock, wait_clock):
        drain_inst = self.nc.sync.drain()
        wait_clock.add_sem_waits(
            drain_inst.ins, ScopedClock({None: tick_clock.global_clock})
        )
        si = drain_inst.ins.sync_info
        waits = list(si.on_wait) if si and si.on_wait else []
        if len(waits) > 1:
            si.on_wait = waits[:1]
            for w in waits[1:]:
                extra = self.nc.sync.drain()
                if extra.ins.sync_info is None:
                    extra.ins.sync_info = mybir.SyncInfo(on_wait=[], on_update=[])
                extra.ins.sync_info.on_wait = [w]

        self.nc.all_engine_barrier()
        assert self.sems is not None
        popped = self.nc._tile_sem_poison_stack.pop()
        assert popped is self._sem_poison
        self.nc.clear_and_free_semaphores(list(self.sems.allocated().values()))
        self.nc.all_engine_barrier()


def build_nc(with_bias=False, debug=False) -> bass.Bass:
    nc = Bacc()
    x = nc.declare_dram_parameter("x", [NB * C, T], F32, isOutput=False)
    wqkvT = nc.declare_dram_parameter("wqkvT", [C, 3 * C], BF16, isOutput=False)
    wprojT = nc.declare_dram_parameter("wprojT", [C, C], BF16, isOutput=False)
    bq = nc.declare_dram_parameter("bq", [P, NQK], F32, isOutput=False)
    bv = nc.declare_dram_parameter("bv", [1, C], BF16, isOutput=False)
    bp = nc.declare_dram_parameter("bp", [P, KC], F32, isOutput=False)
    out = nc.declare_dram_parameter("out", [NB * C, T], F32, isOutput=True)

    with SplitDrainTileContext(nc) as tc, ExitStack() as ctx:
        const = ctx.enter_context(tc.tile_pool(name="const", bufs=1))
        xin = ctx.enter_context(tc.tile_pool(name="xin", bufs=4))
        stat = ctx.enter_context(tc.tile_pool(name="stat", bufs=8))
        xnbp = ctx.enter_context(tc.tile_pool(name="xnb", bufs=2 * KC))
        qkvp = ctx.enter_context(tc.tile_pool(name="qkv", bufs=2 * NQK))
        vtp = ctx.enter_context(tc.tile_pool(name="vt", bufs=16))
        wexpp = ctx.enter_context(tc.tile_pool(name="wexp", bufs=18))
        aallp = ctx.enter_context(tc.tile_pool(name="aall", bufs=2 * KC))
        rbp = ctx.enter_context(tc.tile_pool(name="rb", bufs=4))
        drp = ctx.enter_context(tc.tile_pool(name="dr", bufs=8))
        outp = ctx.enter_context(tc.tile_pool(name="outp", bufs=2))

        # shared psum pool: QK pair tiles + filler unit psums (3 x 2 banks)
        qkf = ctx.enter_context(tc.tile_pool(name="qkf", bufs=3, space="PSUM"))
        # AV accumulates per t-half (1 bank each)
        av_ps = ctx.enter_context(tc.tile_pool(name="avps", bufs=2, space="PSUM"))

        # ---- input DMA: x(b0) chunks interleaved with wqkv chunks so the
        # LN pipeline and the first qkv units both start early ----
        xts = {}
        wq_t = [None] * KC

        def dma_x(b, c):
            xt = xin.tile([P, T], F32, tag="xin", name=f"xin_{b}_{c}")
            nc.sync.dma_start(
                out=xt[:], in_=x[b * C + c * P : b * C + (c + 1) * P, :]
            )
            xts[(b, c)] = xt

        def dma_wq(c):
            t_ = const.tile([P, 3 * C], BF16, tag=f"wq{c}", name=f"wq{c}")
            nc.sync.dma_start(out=t_[:], in_=wqkvT[c * P : (c + 1) * P, :])
            wq_t[c] = t_

        dma_x(0, 0)
        dma_wq(0)
        dma_x(0, 1)
        dma_x(0, 2)
        dma_wq(1)
        dma_x(0, 3)
        dma_wq(2)
        dma_wq(3)
        if with_bias:
            bq_t = const.tile([P, NQK], F32, tag="bq")
            nc.sync.dma_start(out=bq_t[:], in_=bq[:])
            bv_t = const.tile([1, C], BF16, tag="bv")
            nc.sync.dma_start(out=bv_t[:], in_=bv[:])
        for c in range(KC):
            dma_x(1, c)
        wp_t = []
        for c in range(KC):
            t_ = const.tile([P, C], BF16, tag=f"wp{c}", name=f"wp{c}")
            nc.sync.dma_start(out=t_[:], in_=wprojT[c * P : (c + 1) * P, :])
            wp_t.append(t_)
        if with_bias:
            bp_t = const.tile([P, KC], F32, tag="bp")
            nc.sync.dma_start(out=bp_t[:], in_=bp[:])
        eps_t = const.tile([P, 1], F32, tag="eps")
        nc.gpsimd.memset(eps_t[:], EPS)
        ones_t = const.tile([P, 8], BF16, tag="ones")
        nc.gpsimd.memset(ones_t[:], 1.0)
        if with_bias:
            onerow_t = const.tile([1, P], BF16, tag="onerow")
            nc.gpsimd.memset(onerow_t[:], 1.0)
        # dummy exp: forces the ACT exp-table load into the input-DMA shadow
        warm_t = stat.tile([P, 1], F32, tag="warm")
        nc.scalar.activation(warm_t[:], eps_t[:], AF.Exp)

        def head_slice(tiles, h):
            off = (h % 2) * CH
            return tiles[h // 2][off : off + CH, :]

        # per-batch state
        xnb_t = [[None] * KC for _ in range(NB)]
        qkv_t = [[None] * NQK for _ in range(NB)]
        vt_t = [[None] * 8 for _ in range(NB)]
        aall_t = [[None] * KC for _ in range(NB)]
        wexp_t = {}   # (b, h) -> list of 8 chunk tiles
        av_tiles = {}  # (b, h, half) -> [65, 512] psum tile

        def emit_ln(b, c):
            xt = xts[(b, c)]
            bns = stat.tile([P, 12], F32, tag="bns", name=f"bns_{b}_{c}")
            for k in range(2):
                nc.vector.bn_stats(
                    bns[:, 6 * k : 6 * (k + 1)], xt[:, 512 * k : 512 * (k + 1)]
                )
            mv = stat.tile([P, 2], F32, tag="mv", name=f"mv_{b}_{c}")
            nc.vector.bn_aggr(mv[:], bns[:])
            std = stat.tile([P, 1], F32, tag="std", name=f"std_{b}_{c}")
            nc.scalar.activation(std[:], mv[:, 1:2], AF.Sqrt, bias=eps_t[:])
            rstd = stat.tile([P, 1], F32, tag="rstd", name=f"rstd_{b}_{c}")
            nc.vector.reciprocal_approx_fast(rstd[:], std[:])
            xnb = xnbp.tile([P, T], BF16, tag="xnb", name=f"xnb_{b}_{c}")
            nc.vector.tensor_scalar(
                xnb[:], xt[:], scalar1=mv[:, 0:1], scalar2=rstd[:],
                op0=ALU.subtract, op1=ALU.mult,
            )
            xnb_t[b][c] = xnb

        def emit_vt_unit(b, s):
            """v^T for spatial chunk s, all 8 heads: [128 t, 8*65] bf16
            with per-head ones columns."""
            pst = qkf.tile([P, T], F32, tag="qk", name=f"vps_{b}_{s}")
            ps = pst[:, 0:C]
            for c in range(KC):
                nc.tensor.matmul(
                    ps,
                    xnb_t[b][c][:, s * P : (s + 1) * P],
                    wq_t[c][:, 2 * C : 3 * C],
                    start=(c == 0),
                    stop=(c == KC - 1) and not with_bias,
                )
            if with_bias:
                nc.tensor.matmul(ps, onerow_t[:], bv_t[:], start=False, stop=True)
            vt = vtp.tile([P, 8 * VW], BF16, tag="vt", name=f"vt_{b}_{s}")
            with tc.high_priority(offset=HPRI // 2):
                nc.vector.tensor_copy(
                    vt[:].rearrange("p (h c) -> p h c", c=VW)[:, :, 0:CH],
                    ps.rearrange("p (h c) -> p h c", c=CH),
                )
                nc.vector.tensor_copy(
                    vt[:].rearrange("p (h c) -> p h c", c=VW)[:, :, CH : CH + 1],
                    ones_t[:].rearrange("p (h c) -> p h c", c=1),
                )
            vt_t[b][s] = vt

        def emit_qkv_unit(b, ot):
            """One q|k output tile [128, T]: 8 matmuls + one evac pass."""
            qt = qkvp.tile([P, T], BF16, tag="qkv", name=f"qkv_{b}_{ot}")
            ps = qkf.tile([P, T], F32, tag="qk", name=f"qps_{b}_{ot}")
            for c in range(KC):
                for half in range(2):
                    nc.tensor.matmul(
                        ps[:, half * 512 : (half + 1) * 512],
                        wq_t[c][:, ot * P : (ot + 1) * P],
                        xnb_t[b][c][:, half * 512 : (half + 1) * 512],
                        start=(c == 0),
                        stop=(c == KC - 1),
                    )
            with tc.high_priority(offset=HPRI // 2):
                if with_bias:
                    nc.vector.tensor_scalar(
                        qt[:], ps[:], scalar1=bq_t[:, ot : ot + 1], scalar2=None,
                        op0=ALU.add,
                    )
                else:
                    nc.vector.tensor_copy(qt[:], ps[:])
            qkv_t[b][ot] = qt

        def emit_qk_pair(b, hA, s):
            """scores chunk s for heads hA/hA+1: matmuls interleaved
            (A0,B0,A1,B1) in disjoint PE row groups; one exp per head.
            Runs at high priority — this is the exp-chain feeder."""
            hB = hA + 1
            q_all, k_all = qkv_t[b][0:4], qkv_t[b][4:8]
            qA, kA = head_slice(q_all, hA), head_slice(k_all, hA)
            qB, kB = head_slice(q_all, hB), head_slice(k_all, hB)
            with tc.high_priority(offset=HPRI):
                pA = qkf.tile([P, T], F32, tag="qk", name=f"qk_{b}_{hA}_{s}")
                pB = qkf.tile([P, T], F32, tag="qk", name=f"qk_{b}_{hB}_{s}")
                for half in range(2):
                    sl = slice(half * 512, (half + 1) * 512)
                    nc.tensor.matmul(
                        pA[:, sl], kA[:, s * P : (s + 1) * P], qA[:, sl],
                        start=True, stop=True,
                    )
                    nc.tensor.matmul(
                        pB[:, sl], kB[:, s * P : (s + 1) * P], qB[:, sl],
                        start=True, stop=True,
                    )
                for h, ps in ((hA, pA), (hB, pB)):
                    we = wexpp.tile(
                        [P, T], BF16, tag="wexp", name=f"we_{b}_{h}_{s}"
                    )
                    nc.scalar.activation(we[:], ps[:], AF.Exp, scale=0.125)
                    wexp_t.setdefault((b, h), []).append(we)

        def head_off(h):
            return h * VW

        def emit_av_q(b, h, half, squad):
            """AV quad: 4 accumulation matmuls for (head, t-half),
            s-chunks 4*squad..4*squad+3."""
            if squad == 0:
                av_tiles[(b, h, half)] = av_ps.tile(
                    [VW, 512], F32, tag="av", name=f"av_{b}_{h}_{half}"
                )
            av = av_tiles[(b, h, half)]
            for sp in range(4 * squad, 4 * squad + 4):
                nc.tensor.matmul(
                    av[:],
                    vt_t[b][sp][:, head_off(h) : head_off(h) + VW],
                    wexp_t[(b, h)][sp][:, half * 512 : (half + 1) * 512],
                    start=(sp == 0),
                    stop=(sp == 7),
                )

        def emit_norm_half(b, h, half):
            """softmax normalization for one t-half: denom row out of psum,
            reciprocal, gpsimd broadcast, then one multiply reading a'
            straight from PSUM into the aall slice (releases the bank)."""
            av = av_tiles[(b, h, half)]
            with tc.high_priority(offset=HPRI // 2):
                draw = drp.tile([1, 512], F32, tag="draw", name=f"dw_{b}_{h}_{half}")
                nc.vector.tensor_copy(draw[:], av[CH : CH + 1, :])
                drow = drp.tile([1, 512], F32, tag="dr", name=f"dr_{b}_{h}_{half}")
                nc.vector.reciprocal_approx_fast(drow[:], draw[:])
                rb = rbp.tile([CH, 512], F32, tag="rb", name=f"rb_{b}_{h}_{half}")
                nc.gpsimd.partition_broadcast(rb[:], drow[:])
            if aall_t[b][0] is None:
                for i in range(KC):
                    aall_t[b][i] = aallp.tile(
                        [P, T], BF16, tag="aall", name=f"aall_{b}_{i}"
                    )
            dest = head_slice(aall_t[b], h)[:, half * 512 : (half + 1) * 512]
            with tc.high_priority(offset=HPRI // 2):
                nc.vector.tensor_tensor(dest, av[0:CH, :], rb[:], op=ALU.mult)
            if half == 1:
                del wexp_t[(b, h)]

        def proj_evac_half(b, ot, ps_half, half):
            o_t = outp.tile([P, 512], F32, tag="outp", name=f"out_{b}_{ot}_{half}")
            sl = slice(half * 512, (half + 1) * 512)
            ctx2 = tc.high_priority(offset=HPRI // 2)
            ctx2.__enter__()
            if with_bias:
                nc.vector.tensor_scalar(
                    o_t[:], ps_half, scalar1=bp_t[:, ot : ot + 1], scalar2=None,
                    op0=ALU.add,
                )
                nc.vector.tensor_tensor(
                    o_t[:], o_t[:], xnb_t[b][ot][:, sl], op=ALU.add
                )
            else:
                nc.vector.scalar_tensor_tensor(
                    o_t[:], ps_half, 1.0, xnb_t[b][ot][:, sl],
                    op0=ALU.mult, op1=ALU.add,
                )
            nc.sync.dma_start(
                out=out[b * C + ot * P : b * C + (ot + 1) * P, sl], in_=o_t[:]
            )
            ctx2.__exit__(None, None, None)

        def proj_cmms(b, ot, pss, cs, start, stop, rows=None):
            """proj matmuls for chunks cs into pss (2 halves); rows
            optionally restricts the contraction to a 64-row slice."""
            r = rows if rows is not None else slice(0, P)
            for c in cs:
                for half in range(2):
                    nc.tensor.matmul(
                        pss[half],
                        wp_t[c][r, ot * P : (ot + 1) * P],
                        aall_t[b][c][r, half * 512 : (half + 1) * 512],
                        start=start and c == cs[0],
                        stop=stop and c == cs[-1],
                    )

        def emit_proj_unit(b, ot):
            pst = qkf.tile([P, T], F32, tag="qk", name=f"prj_{b}_{ot}")
            pss = [pst[:, 0:512], pst[:, 512:T]]
            proj_cmms(b, ot, pss, list(range(KC)), start=True, stop=True)
            for half in range(2):
                proj_evac_half(b, ot, pss[half], half)

        # ---------------- pipelined schedule ----------------
        for c in range(KC):
            emit_ln(0, c)
        # the first pair's q|k tiles + half the vt units before the loop
        emit_qkv_unit(0, 0)
        emit_qkv_unit(0, 4)
        for s in range(4):
            emit_vt_unit(0, s)

        fillers = (
            [("vt", 0, s) for s in range(4, 8)]
            + [("qkv", 0, 1), ("qkv", 0, 5), ("qkv", 0, 2), ("qkv", 0, 6),
               ("qkv", 0, 3), ("qkv", 0, 7)]
            + [("ln", 1, c) for c in range(KC)]
            + [("qkv", 1, 0), ("qkv", 1, 4), ("qkv", 1, 1), ("qkv", 1, 5),
               ("qkv", 1, 2), ("qkv", 1, 6), ("qkv", 1, 3), ("qkv", 1, 7)]
            + [("vt", 1, s) for s in range(8)]
        )
        proj_units = [(0, ot) for ot in range(KC)]

        def pop_filler(allow_proj):
            if fillers:
                kind, fb, fo = fillers.pop(0)
                if kind == "ln":
                    emit_ln(fb, fo)
                elif kind == "vt":
                    emit_vt_unit(fb, fo)
                else:
                    emit_qkv_unit(fb, fo)
                return True
            if allow_proj and proj_units:
                pb, po = proj_units.pop(0)
                emit_proj_unit(pb, po)
                return True
            return False

        # AV weave for the previous pair: head A's 4 quads over s=0..3
        # (norm per half as it completes), head B over s=4..7.
        def weave_av(pb, pA, s):
            h = pA if s < 4 else pA + 1
            sq = s % 4
            emit_av_q(pb, h, sq // 2, sq % 2)
            if sq == 1:
                emit_norm_half(pb, h, 0)
            elif sq == 3:
                emit_norm_half(pb, h, 1)

        pairs = [(b, 2 * i) for b in range(NB) for i in range(NH // 2)]
        prevp = None
        for pi, (b, hA) in enumerate(pairs):
            for s in range(8):
                emit_qk_pair(b, hA, s)
                if prevp is not None:
                    if pi <= 4 and s in (0, 1, 3, 5, 7):
                        pop_filler(allow_proj=False)
                    elif pi >= 5 and s in (0, 1):
                        pop_filler(allow_proj=False)
                    weave_av(prevp[0], prevp[1], s)
                    if pi >= 5 and s in (2, 6):
                        pop_filler(allow_proj=True)
                else:
                    pop_filler(allow_proj=False)
            prevp = (b, hA)

        # ---------------- drain ----------------
        # AV + norm for the last pair (b1 heads 6,7) with proj(b1) woven in:
        # ot0..2 pre-accumulate c0..2 in the freed shared-pool tiles; their
        # c3 matmuls split into head-6-rows / head-7-rows pieces that chase
        # the final norms; ot3 rides the freed AV banks per half.
        pb, pA = prevp
        h6, h7 = pA, pA + 1
        pre = {}

        def proj_pre(ot):
            pst = qkf.tile([P, T], F32, tag="qk", name=f"prj1_{ot}")
            pss = [pst[:, 0:512], pst[:, 512:T]]
            proj_cmms(1, ot, pss, [0, 1, 2], start=True, stop=False)
            pre[ot] = pss

        def c3_piece(ot, half, rows, stop):
            nc.tensor.matmul(
                pre[ot][half],
                wp_t[3][rows, ot * P : (ot + 1) * P],
                aall_t[1][3][rows, half * 512 : (half + 1) * 512],
                start=False,
                stop=stop,
            )

        while fillers or proj_units:
            pop_filler(allow_proj=True)

        emit_av_q(1, h6, 0, 0); proj_pre(0)
        emit_av_q(1, h6, 0, 1); emit_norm_half(1, h6, 0); proj_pre(1)
        emit_av_q(1, h6, 1, 0); proj_pre(2)
        emit_av_q(1, h6, 1, 1); emit_norm_half(1, h6, 1)

        emit_av_q(1, h7, 0, 0)
        for ot in (0, 1, 2):   # head-6 rows, half 0 (needs norm(6,0))
            c3_piece(ot, 0, slice(0, CH), stop=False)
        emit_av_q(1, h7, 0, 1)
        emit_norm_half(1, h7, 0)
        for ot in (0, 1, 2):   # head-6 rows, half 1
            c3_piece(ot, 1, slice(0, CH), stop=False)
        # ot3 half0 rides the AV bank freed by norm(7,0)
        ps3 = [None, None]
        ps3[0] = av_ps.tile([P, 512], F32, tag="av", name="prj3_0")
        for c in range(3):
            nc.tensor.matmul(
                ps3[0], wp_t[c][:, 3 * P : 4 * P],
                aall_t[1][c][:, 0:512], start=(c == 0), stop=False,
            )
        emit_av_q(1, h7, 1, 0)
        for ot in (0, 1, 2):   # head-7 rows, half 0 (needs norm(7,0))
            c3_piece(ot, 0, slice(CH, P), stop=True)
            proj_evac_half(1, ot, pre[ot][0], 0)
        nc.tensor.matmul(      # ot3 half0 c3 (full contraction)
            ps3[0], wp_t[3][:, 3 * P : 4 * P],
            aall_t[1][3][:, 0:512], start=False, stop=True,
        )
        proj_evac_half(1, 3, ps3[0], 0)
        emit_av_q(1, h7, 1, 1)
        emit_norm_half(1, h7, 1)
        ps3[1] = av_ps.tile([P, 512], F32, tag="av", name="prj3_1")
        for c in range(3):
            nc.tensor.matmul(
                ps3[1], wp_t[c][:, 3 * P : 4 * P],
                aall_t[1][c][:, 512:T], start=(c == 0), stop=False,
            )
        for ot in (0, 1, 2):   # head-7 rows, half 1 — the true tail
            c3_piece(ot, 1, slice(CH, P), stop=True)
            proj_evac_half(1, ot, pre[ot][1], 1)
        nc.tensor.matmul(
            ps3[1], wp_t[3][:, 3 * P : 4 * P],
            aall_t[1][3][:, 512:T], start=False, stop=True,
        )
        proj_evac_half(1, 3, ps3[1], 1)

    nc.finalize()
    return nc


def qkv_perm():
    """Output-channel permutation: legacy [h][q|k|v] interleave -> head-major
    q_all (512) | k_all (512) | v_all (512)."""
    idx = []
    for part in range(3):
        for h in range(NH):
            idx.append(192 * h + part * CH + np.arange(CH))
    return np.concatenate(idx)


def shard_inputs(x, w_qkv, b_qkv, w_proj, b_proj, n_cores=8):
    """Full inputs -> per-core in_maps."""
    import ml_dtypes

    perm = qkv_perm()
    xr = np.ascontiguousarray(x.reshape(16, C, T), dtype=np.float32)
    wqkvT = np.ascontiguousarray(w_qkv[perm].T.astype(ml_dtypes.bfloat16))
    wprojT = np.ascontiguousarray(w_proj.T.astype(ml_dtypes.bfloat16))
    bqp = np.asarray(b_qkv)[perm]
    bqm = np.ascontiguousarray(bqp[: 2 * C].reshape(NQK, P).T, dtype=np.float32)
    bvm = np.ascontiguousarray(bqp[2 * C :].reshape(1, C).astype(ml_dtypes.bfloat16))
    bpm = np.ascontiguousarray(np.asarray(b_proj).reshape(KC, P).T, dtype=np.float32)
    in_maps = []
    for i in range(n_cores):
        in_maps.append(
            {
                "x": np.ascontiguousarray(xr[NB * i : NB * (i + 1)].reshape(NB * C, T)),
                "wqkvT": wqkvT,
                "wprojT": wprojT,
                "bq": bqm,
                "bv": bvm,
                "bp": bpm,
            }
        )
    return in_maps


def gather_outputs(results, n_cores=8):
    outs = [results[i]["out"].reshape(NB, C, 32, 32) for i in range(n_cores)]
    return np.concatenate(outs, axis=0)


# ---------------------------------------------------------------------------
# Cached 8-core PJRT executor (mirrors concourse.bass2jax.run_bass_via_pjrt,
# but keeps the jitted sharded callable alive so repeat kernel() calls skip
# retracing/recompiling)
# ---------------------------------------------------------------------------
import jax
from jax.sharding import Mesh, PartitionSpec

from concourse import bass2jax


def _shard_map():
    try:
        from jax.experimental.shard_map import shard_map
        return shard_map
    except ImportError:
        from jax.experimental import shard_map as sm
        return sm.shard_map


class _Runner:
    def __init__(self, nc, n_cores=8):
        bass2jax.install_neuronx_cc_hook()
        self.nc = nc
        self.n_cores = n_cores
        partition_name = (
            nc.partition_id_tensor.name if nc.partition_id_tensor else None
        )
        in_names, out_names, out_avals, zero_outs = [], [], [], []
        for alloc in nc.m.functions[0].allocations:
            if not isinstance(alloc, mybir.MemoryLocationSet):
                continue
            name = alloc.memorylocations[0].name
            if alloc.kind == "ExternalInput":
                if name != partition_name:
                    in_names.append(name)
            elif alloc.kind == "ExternalOutput":
                shape = tuple(alloc.tensor_shape)
                dtype = mybir.dt.np(alloc.dtype)
                out_names.append(name)
                out_avals.append(jax.core.ShapedArray(shape, dtype))
                zero_outs.append(np.zeros(shape, dtype))
        self.n_params = len(in_names)
        self.out_names = out_names
        self.out_avals = out_avals
        self.zero_outs = zero_outs
        n_outs = len(out_avals)
        in_names = in_names + out_names
        if partition_name is not None:
            in_names.append(partition_name)
        self.in_names = in_names

        def _body(*args):
            operands = list(args)
            if partition_name is not None:
                operands.append(bass2jax.partition_id_tensor())
            outs = bass2jax._bass_exec_p.bind(
                *operands,
                out_avals=tuple(out_avals),
                in_names=tuple(in_names),
                out_names=tuple(out_names),
                lowering_input_output_aliases=(),
                sim_require_finite=True,
                sim_require_nnan=True,
                nc=nc,
            )
            return tuple(outs)

        devices = jax.devices()[:n_cores]
        self.mesh = Mesh(np.asarray(devices), ("core",))
        shard_map = _shard_map()
        in_specs = (PartitionSpec("core"),) * (self.n_params + n_outs)
        out_specs = (PartitionSpec("core"),) * n_outs
        self.sharded = jax.jit(
            shard_map(
                _body,
                mesh=self.mesh,
                in_specs=in_specs,
                out_specs=out_specs,
                check_rep=False,
            ),
            keep_unused=True,
        )

    def run(self, in_maps):
        per_core = [
            [np.asarray(m[name]) for name in self.in_names[: self.n_params]]
            for m in in_maps
        ]
        concat_in = [
            np.concatenate([per_core[c][i] for c in range(self.n_cores)], axis=0)
            for i in range(self.n_params)
        ]
        concat_zeros = [
            np.zeros((self.n_cores * z.shape[0], *z.shape[1:]), z.dtype)
            for z in self.zero_outs
        ]
        out_arrs = self.sharded(*concat_in, *concat_zeros)
        jax.block_until_ready(out_arrs)
        return [
            {
                name: np.asarray(out_arrs[i]).reshape(
                    self.n_cores, *self.out_avals[i].shape
                )[c]
                for i, name in enumerate(self.out_names)
            }
            for c in range(self.n_cores)
        ]


_RUNNERS = {}


def _get_runner(with_bias=False):
    if with_bias not in _RUNNERS:
        _RUNNERS[with_bias] = _Runner(build_nc(with_bias=with_bias), 8)
    return _RUNNERS[with_bias]


def kernel(x, w_qkv, b_qkv, w_proj, b_proj):
    """Full-input AttentionBlock forward on 8 TRN2 NeuronCores.

    x [16, 512, 32, 32] f32 -> out [16, 512, 32, 32] f32.
    Data-parallel over batch: core i computes batches 2i, 2i+1.
    """
    with_bias = bool(np.any(np.asarray(b_qkv))) or bool(np.any(np.asarray(b_proj)))
    runner = _get_runner(with_bias)
    in_maps = shard_inputs(x, w_qkv, b_qkv, w_proj, b_proj, 8)
    results = runner.run(in_maps)
    return gather_outputs(results, 8).astype(np.float32)


# revision 13
# speedup vs baseline: 1.0332x; 1.0181x over previous
"""AttentionBlock Bass kernel for TRN2 — per-core program builder (v6).

Per core: 2 batches of x [512, 1024] (C=512 channels, T=1024 spatial).
Pipeline: layernorm (spatial) -> qkv 1x1 conv -> 8-head attention -> proj
-> residual add.  Matmuls in bf16 (1 cyc/row), accumulation fp32 in PSUM.

The softmax exp chain on the scalar/ACT engine (128 ACTIVATEs of
[128, 1024] ~= 142us) is the pacing critical path; everything else is
arranged to keep it dense:
  - a dummy exp right after the constant memsets forces the ACT exp
    table load into the DMA shadow instead of mid-kernel.
  - qkv units for the first head pair (ot 0 and 4) run before the pair
    loop; all other qkv/vt/LN units are fillers inside the loop.
  - QK pair matmuls emitted interleaved (A0,B0,A1,B1) at base
    partitions 0/64 (disjoint PE row groups) and, with their exps,
    under tc.high_priority so the scheduler never lets filler matmuls
    delay the next exp.
  - PSUM: one shared 3-buffer pool of [128,1024] tiles serves the QK
    pairs AND the filler units (qkv/vt/proj), so the next pair's QK can
    start before the previous pair's last exp completes; AV accumulates
    in two [65,512] half-tiles (1 bank each).
  - softmax denominator folded into AV via a ones-column on v^T (M=65);
    normalization per t-half: psum row -> reciprocal -> gpsimd
    partition-broadcast -> single multiply reading a' straight from
    PSUM (no staging copy).
  - LN uses bn_stats/bn_aggr on DVE (no Square pass on the ACT engine).
  - drain: proj(b1) units pre-accumulate c-chunks 0..2 while the last
    heads' AV runs; the c=3 matmuls are split into 64-contraction
    halves (heads 6 rows / head 7 rows) so after each final norm only
    a handful of matmuls + the evac trail, and the PE never idles past
    the HAM window.

Host-side layouts (see shard_inputs):
  x/out DRAM  [2*512, 1024]   row = b*512 + c
  wqkvT DRAM  [512, 1536]     bf16, output channels permuted head-major
                              q_all|k_all|v_all (qkv_perm)
  bq DRAM     [128, 8]        f32, q|k bias columns per 128-row tile
  bv DRAM     [1, 512]        bf16, v bias row (head-major)
  wprojT DRAM [512, 512]      bf16
  bp DRAM     [128, 4]        f32
"""

import numpy as np
from contextlib import ExitStack

import concourse.bass as bass
import concourse.mybir as mybir
from concourse.bacc import Bacc
from concourse.tile import TileContext
from bass_rust import ScopedClock

F32 = mybir.dt.float32
BF16 = mybir.dt.bfloat16
AF = mybir.ActivationFunctionType
ALU = mybir.AluOpType
AX = mybir.AxisListType

P = 128
T = 1024
NB = 2
C = 512
NH = 8
CH = 64
KC = C // P         # 4 contraction chunks
NQK = (2 * C) // P  # 8 q|k output tiles
EPS = 1e-5
VW = CH + 1         # per-head v^T block width (ones column folded in)
HPRI = 50000        # priority offset for the QK/exp critical chain


class SplitDrainTileContext(TileContext):
    """Kernel-tail drain split into 1-wait chunks (this walrus rejects >1
    sync wait per SP CTRL instruction)."""

    def _drain_and_barrier(self, tick_clock, wait_clock):
        drain_inst = self.nc.sync.drain()
        wait_clock.add_sem_waits(
            drain_inst.ins, ScopedClock({None: tick_clock.global_clock})
        )
        si = drain_inst.ins.sync_info
        waits = list(si.on_wait) if si and si.on_wait else []
        if len(waits) > 1:
            si.on_wait = waits[:1]
            for w in waits[1:]:
                extra = self.nc.sync.drain()
                if extra.ins.sync_info is None:
                    extra.ins.sync_info = mybir.SyncInfo(on_wait=[], on_update=[])
                extra.ins.sync_info.on_wait = [w]

        self.nc.all_engine_barrier()
        assert self.sems is not None
        popped = self.nc._tile_sem_poison_stack.pop()
        assert popped is self._sem_poison
        self.nc.clear_and_free_semaphores(list(self.sems.allocated().values()))
        self.nc.all_engine_barrier()


def build_nc(with_bias=False, debug=False) -> bass.Bass:
    nc = Bacc()
    x = nc.declare_dram_parameter("x", [NB * C, T], F32, isOutput=False)
    wqkvT = nc.declare_dram_parameter("wqkvT", [C, 3 * C], BF16, isOutput=False)
    wprojT = nc.declare_dram_parameter("wprojT", [C, C], BF16, isOutput=False)
    bq = nc.declare_dram_parameter("bq", [P, NQK], F32, isOutput=False)
    bv = nc.declare_dram_parameter("bv", [1, C], BF16, isOutput=False)
    bp = nc.declare_dram_parameter("bp", [P, KC], F32, isOutput=False)
    out = nc.declare_dram_parameter("out", [NB * C, T], F32, isOutput=True)

    with SplitDrainTileContext(nc) as tc, ExitStack() as ctx:
        const = ctx.enter_context(tc.tile_pool(name="const", bufs=1))
        xin = ctx.enter_context(tc.tile_pool(name="xin", bufs=4))
        stat = ctx.enter_context(tc.tile_pool(name="stat", bufs=24))
        xnbp = ctx.enter_context(tc.tile_pool(name="xnb", bufs=2 * KC))
        qkvp = ctx.enter_context(tc.tile_pool(name="qkv", bufs=2 * NQK))
        vtp = ctx.enter_context(tc.tile_pool(name="vt", bufs=16))
        wexpp = ctx.enter_context(tc.tile_pool(name="wexp", bufs=18))
        aallp = ctx.enter_context(tc.tile_pool(name="aall", bufs=2 * KC))
        rbp = ctx.enter_context(tc.tile_pool(name="rb", bufs=4))
        drp = ctx.enter_context(tc.tile_pool(name="dr", bufs=8))
        outp = ctx.enter_context(tc.tile_pool(name="outp", bufs=2))

        # shared psum pool: QK pair tiles + filler unit psums (3 x 2 banks)
        qkf = ctx.enter_context(tc.tile_pool(name="qkf", bufs=3, space="PSUM"))
        # AV accumulates per t-half (1 bank each)
        av_ps = ctx.enter_context(tc.tile_pool(name="avps", bufs=2, space="PSUM"))

        # ---- input DMA: x(b0) chunks interleaved with wqkv chunks so the
        # LN pipeline and the first qkv units both start early ----
        xts = {}
        wq_t = [None] * KC

        def dma_x(b, c):
            xt = xin.tile([P, T], F32, tag="xin", name=f"xin_{b}_{c}")
            nc.sync.dma_start(
                out=xt[:], in_=x[b * C + c * P : b * C + (c + 1) * P, :]
            )
            xts[(b, c)] = xt

        def dma_wq(c):
            t_ = const.tile([P, 3 * C], BF16, tag=f"wq{c}", name=f"wq{c}")
            nc.sync.dma_start(out=t_[:], in_=wqkvT[c * P : (c + 1) * P, :])
            wq_t[c] = t_

        dma_x(0, 0)
        dma_wq(0)
        dma_x(0, 1)
        dma_x(0, 2)
        dma_wq(1)
        dma_x(0, 3)
        dma_wq(2)
        dma_wq(3)
        if with_bias:
            bq_t = const.tile([P, NQK], F32, tag="bq")
            nc.sync.dma_start(out=bq_t[:], in_=bq[:])
            bv_t = const.tile([1, C], BF16, tag="bv")
            nc.sync.dma_start(out=bv_t[:], in_=bv[:])
        for c in range(KC):
            dma_x(1, c)
        wp_t = []
        for c in range(KC):
            t_ = const.tile([P, C], BF16, tag=f"wp{c}", name=f"wp{c}")
            nc.sync.dma_start(out=t_[:], in_=wprojT[c * P : (c + 1) * P, :])
            wp_t.append(t_)
        if with_bias:
            bp_t = const.tile([P, KC], F32, tag="bp")
            nc.sync.dma_start(out=bp_t[:], in_=bp[:])
        eps_t = const.tile([P, 1], F32, tag="eps")
        nc.gpsimd.memset(eps_t[:], EPS)
        ones_t = const.tile([P, 8], BF16, tag="ones")
        nc.gpsimd.memset(ones_t[:], 1.0)
        if with_bias:
            onerow_t = const.tile([1, P], BF16, tag="onerow")
            nc.gpsimd.memset(onerow_t[:], 1.0)

        def head_slice(tiles, h):
            off = (h % 2) * CH
            return tiles[h // 2][off : off + CH, :]

        # per-batch state
        xnb_t = [[None] * KC for _ in range(NB)]
        qkv_t = [[None] * NQK for _ in range(NB)]
        vt_t = [[None] * 8 for _ in range(NB)]
        aall_t = [[None] * KC for _ in range(NB)]
        wexp_t = {}   # (b, h) -> list of 8 chunk tiles
        av_tiles = {}  # (b, h, half) -> [65, 512] psum tile

        ln_stats = {}

        def emit_ln_stats(b, c):
            # all ACT-engine Sqrts happen pre-chain so the exp/sqrt table
            # sets never swap inside the exp stream
            xt = xts[(b, c)]
            bns = stat.tile([P, 12], F32, tag="bns", name=f"bns_{b}_{c}")
            for k in range(2):
                nc.vector.bn_stats(
                    bns[:, 6 * k : 6 * (k + 1)], xt[:, 512 * k : 512 * (k + 1)]
                )
            mv = stat.tile([P, 2], F32, tag="mv", name=f"mv_{b}_{c}")
            nc.vector.bn_aggr(mv[:], bns[:])
            std = stat.tile([P, 1], F32, tag="std", name=f"std_{b}_{c}")
            nc.scalar.activation(std[:], mv[:, 1:2], AF.Sqrt, bias=eps_t[:])
            rstd = stat.tile([P, 1], F32, tag="rstd", name=f"rstd_{b}_{c}")
            nc.vector.reciprocal_approx_fast(rstd[:], std[:])
            ln_stats[(b, c)] = (mv, rstd)

        def emit_ln_evac(b, c):
            mv, rstd = ln_stats[(b, c)]
            xnb = xnbp.tile([P, T], BF16, tag="xnb", name=f"xnb_{b}_{c}")
            nc.vector.tensor_scalar(
                xnb[:], xts[(b, c)][:], scalar1=mv[:, 0:1], scalar2=rstd[:],
                op0=ALU.subtract, op1=ALU.mult,
            )
            xnb_t[b][c] = xnb

        def emit_ln(b, c):
            emit_ln_stats(b, c)
            emit_ln_evac(b, c)

        def emit_vt_unit(b, s):
            """v^T for spatial chunk s, all 8 heads: [128 t, 8*65] bf16
            with per-head ones columns."""
            pst = qkf.tile([P, T], F32, tag="qk", name=f"vps_{b}_{s}")
            ps = pst[:, 0:C]
            for c in range(KC):
                nc.tensor.matmul(
                    ps,
                    xnb_t[b][c][:, s * P : (s + 1) * P],
                    wq_t[c][:, 2 * C : 3 * C],
                    start=(c == 0),
                    stop=(c == KC - 1) and not with_bias,
                )
            if with_bias:
                nc.tensor.matmul(ps, onerow_t[:], bv_t[:], start=False, stop=True)
            vt = vtp.tile([P, 8 * VW], BF16, tag="vt", name=f"vt_{b}_{s}")
            with tc.high_priority(offset=HPRI // 2):
                nc.vector.tensor_copy(
                    vt[:].rearrange("p (h c) -> p h c", c=VW)[:, :, 0:CH],
                    ps.rearrange("p (h c) -> p h c", c=CH),
                )
                nc.vector.tensor_copy(
                    vt[:].rearrange("p (h c) -> p h c", c=VW)[:, :, CH : CH + 1],
                    ones_t[:].rearrange("p (h c) -> p h c", c=1),
                )
            vt_t[b][s] = vt

        def emit_qkv_unit(b, ot):
            """One q|k output tile [128, T]: 8 matmuls + one evac pass."""
            qt = qkvp.tile([P, T], BF16, tag="qkv", name=f"qkv_{b}_{ot}")
            ps = qkf.tile([P, T], F32, tag="qk", name=f"qps_{b}_{ot}")
            for c in range(KC):
                for half in range(2):
                    nc.tensor.matmul(
                        ps[:, half * 512 : (half + 1) * 512],
                        wq_t[c][:, ot * P : (ot + 1) * P],
                        xnb_t[b][c][:, half * 512 : (half + 1) * 512],
                        start=(c == 0),
                        stop=(c == KC - 1),
                    )
            with tc.high_priority(offset=HPRI // 2):
                if with_bias:
                    nc.vector.tensor_scalar(
                        qt[:], ps[:], scalar1=bq_t[:, ot : ot + 1], scalar2=None,
                        op0=ALU.add,
                    )
                else:
                    nc.vector.tensor_copy(qt[:], ps[:])
            qkv_t[b][ot] = qt

        def emit_qk_pair(b, hA, s):
            """scores chunk s for heads hA/hA+1: matmuls interleaved
            (A0,B0,A1,B1) in disjoint PE row groups; one exp per head.
            Runs at high priority — this is the exp-chain feeder."""
            hB = hA + 1
            q_all, k_all = qkv_t[b][0:4], qkv_t[b][4:8]
            qA, kA = head_slice(q_all, hA), head_slice(k_all, hA)
            qB, kB = head_slice(q_all, hB), head_slice(k_all, hB)
            with tc.high_priority(offset=HPRI):
                pA = qkf.tile([P, T], F32, tag="qk", name=f"qk_{b}_{hA}_{s}")
                pB = qkf.tile([P, T], F32, tag="qk", name=f"qk_{b}_{hB}_{s}")
                for half in range(2):
                    sl = slice(half * 512, (half + 1) * 512)
                    nc.tensor.matmul(
                        pA[:, sl], kA[:, s * P : (s + 1) * P], qA[:, sl],
                        start=True, stop=True,
                    )
                    nc.tensor.matmul(
                        pB[:, sl], kB[:, s * P : (s + 1) * P], qB[:, sl],
                        start=True, stop=True,
                    )
                for h, ps in ((hA, pA), (hB, pB)):
                    we = wexpp.tile(
                        [P, T], BF16, tag="wexp", name=f"we_{b}_{h}_{s}"
                    )
                    nc.scalar.activation(we[:], ps[:], AF.Exp, scale=0.125)
                    wexp_t.setdefault((b, h), []).append(we)

        def head_off(h):
            return h * VW

        def emit_av_q(b, h, half, squad):
            """AV quad: 4 accumulation matmuls for (head, t-half),
            s-chunks 4*squad..4*squad+3."""
            if squad == 0:
                av_tiles[(b, h, half)] = av_ps.tile(
                    [VW, 512], F32, tag="av", name=f"av_{b}_{h}_{half}"
                )
            av = av_tiles[(b, h, half)]
            for sp in range(4 * squad, 4 * squad + 4):
                nc.tensor.matmul(
                    av[:],
                    vt_t[b][sp][:, head_off(h) : head_off(h) + VW],
                    wexp_t[(b, h)][sp][:, half * 512 : (half + 1) * 512],
                    start=(sp == 0),
                    stop=(sp == 7),
                )

        def emit_norm_half(b, h, half):
            """softmax normalization for one t-half: denom row out of psum,
            reciprocal, gpsimd broadcast, then one multiply reading a'
            straight from PSUM into the aall slice (releases the bank)."""
            av = av_tiles[(b, h, half)]
            with tc.high_priority(offset=HPRI // 2):
                draw = drp.tile([1, 512], F32, tag="draw", name=f"dw_{b}_{h}_{half}")
                nc.vector.tensor_copy(draw[:], av[CH : CH + 1, :])
                drow = drp.tile([1, 512], F32, tag="dr", name=f"dr_{b}_{h}_{half}")
                nc.vector.reciprocal_approx_fast(drow[:], draw[:])
                rb = rbp.tile([CH, 512], F32, tag="rb", name=f"rb_{b}_{h}_{half}")
                nc.gpsimd.partition_broadcast(rb[:], drow[:])
            if aall_t[b][0] is None:
                for i in range(KC):
                    aall_t[b][i] = aallp.tile(
                        [P, T], BF16, tag="aall", name=f"aall_{b}_{i}"
                    )
            dest = head_slice(aall_t[b], h)[:, half * 512 : (half + 1) * 512]
            with tc.high_priority(offset=HPRI // 2):
                nc.vector.tensor_tensor(dest, av[0:CH, :], rb[:], op=ALU.mult)
            if half == 1:
                del wexp_t[(b, h)]

        def proj_evac_half(b, ot, ps_half, half):
            o_t = outp.tile([P, 512], F32, tag="outp", name=f"out_{b}_{ot}_{half}")
            sl = slice(half * 512, (half + 1) * 512)
            ctx2 = tc.high_priority(offset=HPRI // 2)
            ctx2.__enter__()
            if with_bias:
                nc.vector.tensor_scalar(
                    o_t[:], ps_half, scalar1=bp_t[:, ot : ot + 1], scalar2=None,
                    op0=ALU.add,
                )
                nc.vector.tensor_tensor(
                    o_t[:], o_t[:], xnb_t[b][ot][:, sl], op=ALU.add
                )
            else:
                nc.vector.scalar_tensor_tensor(
                    o_t[:], ps_half, 1.0, xnb_t[b][ot][:, sl],
                    op0=ALU.mult, op1=ALU.add,
                )
            nc.sync.dma_start(
                out=out[b * C + ot * P : b * C + (ot + 1) * P, sl], in_=o_t[:]
            )
            ctx2.__exit__(None, None, None)

        def proj_cmms(b, ot, pss, cs, start, stop, rows=None):
            """proj matmuls for chunks cs into pss (2 halves); rows
            optionally restricts the contraction to a 64-row slice."""
            r = rows if rows is not None else slice(0, P)
            for c in cs:
                for half in range(2):
                    nc.tensor.matmul(
                        pss[half],
                        wp_t[c][r, ot * P : (ot + 1) * P],
                        aall_t[b][c][r, half * 512 : (half + 1) * 512],
                        start=start and c == cs[0],
                        stop=stop and c == cs[-1],
                    )

        def emit_proj_unit(b, ot):
            pst = qkf.tile([P, T], F32, tag="qk", name=f"prj_{b}_{ot}")
            pss = [pst[:, 0:512], pst[:, 512:T]]
            proj_cmms(b, ot, pss, list(range(KC)), start=True, stop=True)
            for half in range(2):
                proj_evac_half(b, ot, pss[half], half)

        # ---------------- pipelined schedule ----------------
        for c in range(KC):
            emit_ln(0, c)
        # the first pair's q|k tiles + half the vt units before the loop;
        # b1's LN stats too, so every Sqrt precedes the first exp
        emit_qkv_unit(0, 0)
        emit_qkv_unit(0, 4)
        for s in range(4):
            emit_vt_unit(0, s)
        for c in range(KC):
            emit_ln_stats(1, c)

        fillers = (
            [("vt", 0, s) for s in range(4, 8)]
            + [("qkv", 0, 1), ("qkv", 0, 5), ("qkv", 0, 2), ("qkv", 0, 6),
               ("qkv", 0, 3), ("qkv", 0, 7)]
            + [("lnE", 1, c) for c in range(KC)]
            + [("qkv", 1, 0), ("qkv", 1, 4), ("qkv", 1, 1), ("qkv", 1, 5),
               ("qkv", 1, 2), ("qkv", 1, 6), ("qkv", 1, 3), ("qkv", 1, 7)]
            + [("vt", 1, s) for s in range(8)]
        )
        proj_units = [(0, ot) for ot in range(KC)]

        def pop_filler(allow_proj):
            if fillers:
                kind, fb, fo = fillers.pop(0)
                if kind == "lnE":
                    emit_ln_evac(fb, fo)
                elif kind == "vt":
                    emit_vt_unit(fb, fo)
                else:
                    emit_qkv_unit(fb, fo)
                return True
            if allow_proj and proj_units:
                pb, po = proj_units.pop(0)
                emit_proj_unit(pb, po)
                return True
            return False

        # AV weave for the previous pair: head A's 4 quads over s=0..3
        # (norm per half as it completes), head B over s=4..7.
        def weave_av(pb, pA, s):
            h = pA if s < 4 else pA + 1
            sq = s % 4
            emit_av_q(pb, h, sq // 2, sq % 2)
            if sq == 1:
                emit_norm_half(pb, h, 0)
            elif sq == 3:
                emit_norm_half(pb, h, 1)

        pairs = [(b, 2 * i) for b in range(NB) for i in range(NH // 2)]
        prevp = None
        for pi, (b, hA) in enumerate(pairs):
            last = pi == len(pairs) - 1
            for s in range(8):
                emit_qk_pair(b, hA, s)
                if prevp is None:
                    pop_filler(allow_proj=False)
                elif last:
                    # compressed: pair6's heads in the first half of the
                    # slots; heads 6,7's AV then rides the remaining exp
                    # windows (emitted at s7 once their wexp tiles exist —
                    # the scheduler hoists them into earlier idle gaps)
                    ph = prevp[1]
                    if s == 0:
                        emit_av_q(1, ph, 0, 0)
                        emit_av_q(1, ph, 0, 1)
                    elif s == 1:
                        emit_norm_half(1, ph, 0)
                        emit_av_q(1, ph, 1, 0)
                        emit_av_q(1, ph, 1, 1)
                    elif s == 2:
                        emit_norm_half(1, ph, 1)
                        emit_av_q(1, ph + 1, 0, 0)
                        emit_av_q(1, ph + 1, 0, 1)
                    elif s == 3:
                        emit_norm_half(1, ph + 1, 0)
                        emit_av_q(1, ph + 1, 1, 0)
                        emit_av_q(1, ph + 1, 1, 1)
                    elif s == 4:
                        emit_norm_half(1, ph + 1, 1)
                        emit_av_q(1, hA, 0, 0)
                    elif s == 5:
                        emit_av_q(1, hA, 1, 0)
                    elif s == 7:
                        emit_av_q(1, hA, 0, 1)
                        emit_norm_half(1, hA, 0)
                        emit_av_q(1, hA, 1, 1)
                        emit_norm_half(1, hA, 1)
                        emit_av_q(1, hA + 1, 0, 0)
                        emit_av_q(1, hA + 1, 0, 1)
                        emit_norm_half(1, hA + 1, 0)
                        emit_av_q(1, hA + 1, 1, 0)
                        emit_av_q(1, hA + 1, 1, 1)
                else:
                    if pi <= 4 and s in (0, 1, 3, 5, 7):
                        pop_filler(allow_proj=False)
                    elif pi >= 5 and s in (0, 1):
                        pop_filler(allow_proj=False)
                    weave_av(prevp[0], prevp[1], s)
                    if pi in (5, 6) and s in (2, 6):
                        pop_filler(allow_proj=True)
            prevp = (b, hA)

        # ---------------- drain ----------------
        # the compressed last-pair weave already ran all AV quads and every
        # norm except (h7, half1).  Left: proj(b1) — ot0..2 pre-accumulate
        # c0..2 in freed shared-pool tiles with c3 split into head-6-row /
        # head-7-row pieces; ot3 rides the freed AV banks per half.
        pb, pA = prevp
        h6, h7 = pA, pA + 1
        pre = {}

        def proj_pre(ot):
            pst = qkf.tile([P, T], F32, tag="qk", name=f"prj1_{ot}")
            pss = [pst[:, 0:512], pst[:, 512:T]]
            proj_cmms(1, ot, pss, [0, 1, 2], start=True, stop=False)
            pre[ot] = pss

        def c3_piece(ot, half, rows, stop):
            nc.tensor.matmul(
                pre[ot][half],
                wp_t[3][rows, ot * P : (ot + 1) * P],
                aall_t[1][3][rows, half * 512 : (half + 1) * 512],
                start=False,
                stop=stop,
            )

        while fillers or proj_units:
            pop_filler(allow_proj=True)

        proj_pre(0)
        proj_pre(1)
        proj_pre(2)
        for half in (0, 1):      # head-6 rows (norms ran in-loop)
            for ot in (0, 1, 2):
                c3_piece(ot, half, slice(0, CH), stop=False)
        emit_norm_half(1, h7, 1)
        for ot in (0, 1, 2):     # head-7 rows, half 0
            c3_piece(ot, 0, slice(CH, P), stop=True)
            proj_evac_half(1, ot, pre[ot][0], 0)
        ps3 = av_ps.tile([P, 512], F32, tag="av", name="prj3_0")
        for c in range(KC):
            nc.tensor.matmul(
                ps3, wp_t[c][:, 3 * P : 4 * P],
                aall_t[1][c][:, 0:512], start=(c == 0), stop=(c == KC - 1),
            )
        proj_evac_half(1, 3, ps3, 0)
        for ot in (0, 1, 2):     # head-7 rows, half 1 — the true tail
            c3_piece(ot, 1, slice(CH, P), stop=True)
            proj_evac_half(1, ot, pre[ot][1], 1)
        ps3b = av_ps.tile([P, 512], F32, tag="av", name="prj3_1")
        for c in range(KC):
            nc.tensor.matmul(
                ps3b, wp_t[c][:, 3 * P : 4 * P],
                aall_t[1][c][:, 512:T], start=(c == 0), stop=(c == KC - 1),
            )
        proj_evac_half(1, 3, ps3b, 1)

    nc.finalize()
    return nc


def qkv_perm():
    """Output-channel permutation: legacy [h][q|k|v] interleave -> head-major
    q_all (512) | k_all (512) | v_all (512)."""
    idx = []
    for part in range(3):
        for h in range(NH):
            idx.append(192 * h + part * CH + np.arange(CH))
    return np.concatenate(idx)


def shard_inputs(x, w_qkv, b_qkv, w_proj, b_proj, n_cores=8):
    """Full inputs -> per-core in_maps."""
    import ml_dtypes

    perm = qkv_perm()
    xr = np.ascontiguousarray(x.reshape(16, C, T), dtype=np.float32)
    wqkvT = np.ascontiguousarray(w_qkv[perm].T.astype(ml_dtypes.bfloat16))
    wprojT = np.ascontiguousarray(w_proj.T.astype(ml_dtypes.bfloat16))
    bqp = np.asarray(b_qkv)[perm]
    bqm = np.ascontiguousarray(bqp[: 2 * C].reshape(NQK, P).T, dtype=np.float32)
    bvm = np.ascontiguousarray(bqp[2 * C :].reshape(1, C).astype(ml_dtypes.bfloat16))
    bpm = np.ascontiguousarray(np.asarray(b_proj).reshape(KC, P).T, dtype=np.float32)
    in_maps = []
    for i in range(n_cores):
        in_maps.append(
            {
                "x": np.ascontiguousarray(xr[NB * i : NB * (i + 1)].reshape(NB * C, T)),
                "wqkvT": wqkvT,
                "wprojT": wprojT,
                "bq": bqm,
                "bv": bvm,
                "bp": bpm,
            }
        )
    return in_maps


def gather_outputs(results, n_cores=8):
    outs = [results[i]["out"].reshape(NB, C, 32, 32) for i in range(n_cores)]
    return np.concatenate(outs, axis=0)


# ---------------------------------------------------------------------------
# Cached 8-core PJRT executor (mirrors concourse.bass2jax.run_bass_via_pjrt,
# but keeps the jitted sharded callable alive so repeat kernel() calls skip
# retracing/recompiling)
# ---------------------------------------------------------------------------
import jax
from jax.sharding import Mesh, PartitionSpec

from concourse import bass2jax


def _shard_map():
    try:
        from jax.experimental.shard_map import shard_map
        return shard_map
    except ImportError:
        from jax.experimental import shard_map as sm
        return sm.shard_map


class _Runner:
    def __init__(self, nc, n_cores=8):
        bass2jax.install_neuronx_cc_hook()
        self.nc = nc
        self.n_cores = n_cores
        partition_name = (
            nc.partition_id_tensor.name if nc.partition_id_tensor else None
        )
        in_names, out_names, out_avals, zero_outs = [], [], [], []
        for alloc in nc.m.functions[0].allocations:
            if not isinstance(alloc, mybir.MemoryLocationSet):
                continue
            name = alloc.memorylocations[0].name
            if alloc.kind == "ExternalInput":
                if name != partition_name:
                    in_names.append(name)
            elif alloc.kind == "ExternalOutput":
                shape = tuple(alloc.tensor_shape)
                dtype = mybir.dt.np(alloc.dtype)
                out_names.append(name)
                out_avals.append(jax.core.ShapedArray(shape, dtype))
                zero_outs.append(np.zeros(shape, dtype))
        self.n_params = len(in_names)
        self.out_names = out_names
        self.out_avals = out_avals
        self.zero_outs = zero_outs
        n_outs = len(out_avals)
        in_names = in_names + out_names
        if partition_name is not None:
            in_names.append(partition_name)
        self.in_names = in_names

        def _body(*args):
            operands = list(args)
            if partition_name is not None:
                operands.append(bass2jax.partition_id_tensor())
            outs = bass2jax._bass_exec_p.bind(
                *operands,
                out_avals=tuple(out_avals),
                in_names=tuple(in_names),
                out_names=tuple(out_names),
                lowering_input_output_aliases=(),
                sim_require_finite=True,
                sim_require_nnan=True,
                nc=nc,
            )
            return tuple(outs)

        devices = jax.devices()[:n_cores]
        self.mesh = Mesh(np.asarray(devices), ("core",))
        shard_map = _shard_map()
        in_specs = (PartitionSpec("core"),) * (self.n_params + n_outs)
        out_specs = (PartitionSpec("core"),) * n_outs
        self.sharded = jax.jit(
            shard_map(
                _body,
                mesh=self.mesh,
                in_specs=in_specs,
                out_specs=out_specs,
                check_rep=False,
            ),
            keep_unused=True,
        )

    def run(self, in_maps):
        per_core = [
            [np.asarray(m[name]) for name in self.in_names[: self.n_params]]
            for m in in_maps
        ]
        concat_in = [
            np.concatenate([per_core[c][i] for c in range(self.n_cores)], axis=0)
            for i in range(self.n_params)
        ]
        concat_zeros = [
            np.zeros((self.n_cores * z.shape[0], *z.shape[1:]), z.dtype)
            for z in self.zero_outs
        ]
        out_arrs = self.sharded(*concat_in, *concat_zeros)
        jax.block_until_ready(out_arrs)
        return [
            {
                name: np.asarray(out_arrs[i]).reshape(
                    self.n_cores, *self.out_avals[i].shape
                )[c]
                for i, name in enumerate(self.out_names)
            }
            for c in range(self.n_cores)
        ]


_RUNNERS = {}


def _get_runner(with_bias=False):
    if with_bias not in _RUNNERS:
        _RUNNERS[with_bias] = _Runner(build_nc(with_bias=with_bias), 8)
    return _RUNNERS[with_bias]


def kernel(x, w_qkv, b_qkv, w_proj, b_proj):
    """Full-input AttentionBlock forward on 8 TRN2 NeuronCores.

    x [16, 512, 32, 32] f32 -> out [16, 512, 32, 32] f32.
    Data-parallel over batch: core i computes batches 2i, 2i+1.
    """
    with_bias = bool(np.any(np.asarray(b_qkv))) or bool(np.any(np.asarray(b_proj)))
    runner = _get_runner(with_bias)
    in_maps = shard_inputs(x, w_qkv, b_qkv, w_proj, b_proj, 8)
    results = runner.run(in_maps)
    return gather_outputs(results, 8).astype(np.float32)
